# revision 13
# baseline (speedup 1.0000x reference)
"""EnhancedLoRALinear Trainium2 kernel.

Computes, for x:[4,8192,1024] and torch-style weights (out,in):
    out = x @ (W + W_res)^T + b + sigmoid(x @ W_gate^T) * (2 * (x @ W_down^T) @ W_up^T)

Strategy:
  - Data-parallel: the 32768 tokens are split across 8 NeuronCores (4096 each);
    the small weight matrices are replicated.
  - Algebraic fold: main + residual share one matmul with Wc = W + W_res.
  - Host prep: weights are pre-transposed to [in, out] so the contraction dim
    lands on SBUF partitions; x shards are pre-transposed to [in, tokens] for
    the same reason. LoRA scaling (2.0, exact in fp32) is folded into W_up^T.
  - Device: fp32r matmuls (full PE rate at moving free dim >= 256). Per
    128-token tile and 512-wide output half: a K=1 ones-row matmul seeds the
    main PSUM with the bias, 8 k-tile matmuls accumulate the main path, 8 the
    gate path, one K=16 matmul applies the LoRA up-projection from an
    [R=16, 512-token] down-projection computed once per 512 tokens. Sigmoid
    runs on ScalarE; gate*lora and +main on VectorE.
  - Sync-wait budget: fp32r matmuls can encode ONE hw sync-wait, other engine
    ops TWO. Hence: each multi-part tensor loads with a single DMA instruction
    (one queue semaphore), warm-up matmuls make the PE observe every weight
    DMA semaphore first (enforced via ordering deps), and the epilogue is
    shaped so every op joins at most two semaphores.
"""

import ml_dtypes
import numpy as np

_BF16 = ml_dtypes.bfloat16

import concourse.bass as bass
import concourse.bacc as bacc
import concourse.mybir as mybir
import concourse.tile as tile
from concourse.bass_utils import run_bass_kernel_spmd
from concourse.tile_rust import add_dep_helper

N_CORES = 8
B, S = 4, 8192
TOK = B * S                  # 32768 tokens total
T = TOK // N_CORES           # 4096 tokens per core
I = 1024                     # in_features
O = 1024                     # out_features
R = 16                       # lora rank
SCALING = 2.0                # lora_alpha / r (exact power of two)
KT = I // 128                # 8 contraction tiles
TG = 512                     # token group (down-projection batch)
NG = T // TG                 # 8 groups per core
NH = O // 512                # 2 output halves

F32 = mybir.dt.float32
F32R = mybir.dt.float32r


def _build_nc():
    nc = bacc.Bacc(None)

    xt = nc.dram_tensor("xt", [I, T], F32R, kind="ExternalInput")
    wct = nc.dram_tensor("wct", [I, O], F32R, kind="ExternalInput")
    wgt = nc.dram_tensor("wgt", [I, O], mybir.dt.bfloat16, kind="ExternalInput")
    xtb = nc.dram_tensor("xtb", [I, T], mybir.dt.bfloat16, kind="ExternalInput")
    wdt = nc.dram_tensor("wdt", [I, R], F32R, kind="ExternalInput")
    wut2 = nc.dram_tensor("wut2", [R, O], F32R, kind="ExternalInput")
    biasbc = nc.dram_tensor("biasbc", [128, O], F32, kind="ExternalInput")
    out = nc.dram_tensor("out", [T, O], F32, kind="ExternalOutput")

    # [i, o] -> [p, k, o] views so each weight loads with ONE DMA instruction
    xt_v = xt.rearrange("(k p) t -> p k t", p=128)
    xtb_v = xtb.rearrange("(k p) t -> p k t", p=128)
    wct_v = wct.rearrange("(k p) o -> p k o", p=128)
    wgt_v = wgt.rearrange("(k p) o -> p k o", p=128)
    wdt_v = wdt.rearrange("(k p) r -> p k r", p=128)

    sig = mybir.ActivationFunctionType.Sigmoid
    mult = mybir.AluOpType.mult
    add = mybir.AluOpType.add

    with tile.TileContext(nc) as tc:
        with (
            tc.tile_pool(name="wpool", bufs=1) as wpool,
            tc.tile_pool(name="xpool", bufs=2) as xpool,
            tc.tile_pool(name="opool", bufs=3) as opool,
            tc.tile_pool(name="epool", bufs=3) as epool,
            tc.tile_pool(name="psum", bufs=1, space="PSUM") as pp,
        ):
            # --- resident weights, one DMA each ---
            wc_sb = wpool.tile([128, KT, O], F32R)
            wg_sb = wpool.tile([128, KT, O], mybir.dt.bfloat16)
            wd_sb = wpool.tile([128, KT, R], F32R)
            wu_sb = wpool.tile([R, O], F32R)
            bias_bc = wpool.tile([128, O], F32)

            for k in range(KT):
                nc.sync.dma_start(out=wc_sb[:, k, :], in_=wct_v[:, k, :])
            nc.sync.dma_start(out=wg_sb[:, 0:4, :], in_=wgt_v[:, 0:4, :])
            nc.sync.dma_start(out=wg_sb[:, 4:8, :], in_=wgt_v[:, 4:8, :])
            nc.sync.dma_start(out=wd_sb[:, :, :], in_=wdt_v[:, :, :])
            nc.sync.dma_start(out=wu_sb[:, :], in_=wut2[:, :])
            nc.sync.dma_start(out=bias_bc[:, :], in_=biasbc[:, :])

            # HAM spin-up: ~60 junk matmuls keep the PE busy through the DMA
            # prologue so the clock gate opens before real compute starts
            junk = wpool.tile([128, 512], mybir.dt.bfloat16)
            nc.gpsimd.memset(junk[:, :], 0.0)
            warm = pp.tile([128, 512], F32, tag="warm")
            spin = None
            for i in range(45):
                spin = nc.tensor.matmul(warm[:, :], junk[:, 0:128], junk[:, :],
                                        start=True, stop=True)

            # warm-up matmuls: make the PE observe each weight-DMA semaphore
            # (fp32r matmuls can encode only one sync-wait downstream)
            warms = [
                nc.tensor.matmul(warm[0:1, :], wc_sb[:, k, 0:1],
                                 wc_sb[:, k, 0:512], start=True, stop=True)
                for k in range(KT)
            ] + [
                nc.tensor.matmul(warm[0:1, :], wg_sb[:, 0, 0:1],
                                 wg_sb[:, 0, 0:512], start=True, stop=True),
                nc.tensor.matmul(warm[0:1, :], wg_sb[:, 4, 0:1],
                                 wg_sb[:, 4, 0:512], start=True, stop=True),
                nc.tensor.matmul(warm[0:16, 0:16], wd_sb[:, 0, :],
                                 wd_sb[:, 0, :], start=True, stop=True),
                nc.tensor.matmul(warm[0:1, :], wu_sb[:, 0:1],
                                 wu_sb[:, 0:512], start=True, stop=True),
                spin,
            ]
            first_real = []  # first matmul of each psum group in group 0

            for g in range(NG):
                tg0 = g * TG
                xt_sb = xpool.tile([128, KT, TG], F32R, tag="xt")
                nc.sync.dma_start(
                    out=xt_sb[:, :, :], in_=xt_v[:, :, tg0 : tg0 + TG]
                )
                xtb_sb = xpool.tile([128, KT, TG], mybir.dt.bfloat16, tag="xtb")
                nc.sync.dma_start(
                    out=xtb_sb[:, :, :], in_=xtb_v[:, :, tg0 : tg0 + TG]
                )

                # LoRA down-projection for the whole 512-token group: [R, TG]
                dps = pp.tile([R, TG], F32, tag="misc")
                for k in range(KT):
                    mm = nc.tensor.matmul(
                        dps[:, :],
                        wd_sb[:, k, :],
                        xt_sb[:, k, :],
                        start=(k == 0),
                        stop=(k == KT - 1),
                    )
                    if g == 0 and k == 0:
                        first_real.append(mm)
                down_sb = epool.tile([R, TG], F32R, tag="down")
                nc.vector.tensor_copy(down_sb[:, :], dps[:, :])

                for t in range(TG // 128):
                    tsl = slice(t * 128, (t + 1) * 128)
                    out_sb = opool.tile([128, O], F32, tag="out")
                    for oh in range(NH):
                        osl = slice(oh * 512, (oh + 1) * 512)
                        mps = pp.tile([128, 512], F32, tag=f"main{oh}")
                        gps = pp.tile([128, 512], F32, tag=f"gate{oh}")
                        lps = pp.tile([128, 512], F32, tag=f"lora{oh}")
                        for k in range(KT):
                            mm = nc.tensor.matmul(
                                mps[:, :],
                                xt_sb[:, k, tsl],
                                wc_sb[:, k, osl],
                                start=(k == 0),
                                stop=(k == KT - 1),
                            )
                            if g == 0 and t == 0 and k == 0:
                                first_real.append(mm)
                        for k in range(KT):
                            mm = nc.tensor.matmul(
                                gps[:, :],
                                xtb_sb[:, k, tsl],
                                wg_sb[:, k, osl],
                                start=(k == 0),
                                stop=(k == KT - 1),
                            )
                            if g == 0 and t == 0 and k == 0:
                                first_real.append(mm)
                        mm = nc.tensor.matmul(
                            lps[:, :],
                            down_sb[:, tsl],
                            wu_sb[:, osl],
                            start=True,
                            stop=True,
                        )
                        if g == 0 and t == 0:
                            first_real.append(mm)
                        g_sb = epool.tile([128, 512], F32, tag="sig")
                        nc.scalar.activation(g_sb[:, :], gps[:, :], sig)
                        gl_sb = epool.tile([128, 512], F32, tag="gl")
                        nc.vector.tensor_tensor(
                            gl_sb[:, :], g_sb[:, :], lps[:, :], mult
                        )
                        nc.gpsimd.tensor_tensor(
                            gl_sb[:, :], gl_sb[:, :], bias_bc[:, osl], add
                        )
                        nc.vector.tensor_tensor(
                            out_sb[:, osl], gl_sb[:, :], mps[:, :], add
                        )
                    nc.sync.dma_start(
                        out=out[tg0 + t * 128 : tg0 + (t + 1) * 128, :],
                        in_=out_sb[:, :],
                    )

            # ordering-only deps: every warm-up precedes the first matmul of
            # each group-0 psum chain, so no real matmul lands before the PE
            # has observed all weight DMA semaphores
            for w in warms:
                for fr in first_real:
                    add_dep_helper(fr.ins, w.ins, False,
                                   "warmups before real matmuls")
    nc.compile()
    return nc


_NC_CACHE = None


def _get_nc():
    global _NC_CACHE
    if _NC_CACHE is None:
        _NC_CACHE = _build_nc()
    return _NC_CACHE


def _prep_inputs(x, W, b, W_down, W_up, W_gate, W_res):
    x = np.asarray(x, dtype=np.float32).reshape(TOK, I)
    wct = np.ascontiguousarray((np.asarray(W) + np.asarray(W_res)).T.astype(np.float32))
    wgt = np.ascontiguousarray(np.asarray(W_gate).T.astype(_BF16))
    wdt = np.ascontiguousarray(np.asarray(W_down).T.astype(np.float32))
    wut2 = np.ascontiguousarray((SCALING * np.asarray(W_up)).T.astype(np.float32))
    biasbc = np.ascontiguousarray(
        np.broadcast_to(np.asarray(b, dtype=np.float32).reshape(1, O), (128, O))
    )
    in_maps = []
    for c in range(N_CORES):
        xt_c = np.ascontiguousarray(x[c * T : (c + 1) * T, :].T)
        xtb_c = xt_c.astype(_BF16)
        in_maps.append(
            {
                "xt": xt_c,
                "xtb": xtb_c,
                "wct": wct,
                "wgt": wgt,
                "wdt": wdt,
                "wut2": wut2,
                "biasbc": biasbc,
            }
        )
    return in_maps


def run(inputs, trace=False, **kwargs):
    """Build + run on the 8 NeuronCores. Returns (full_output, BassKernelResults)."""
    nc = _get_nc()
    in_maps = _prep_inputs(**inputs)
    res = run_bass_kernel_spmd(
        nc, in_maps, list(range(N_CORES)), trace=trace, **kwargs
    )
    shards = [res.results[c]["out"] for c in range(N_CORES)]
    full = np.concatenate(shards, axis=0).reshape(B, S, O)
    return full, res


def kernel(**inputs):
    out, _ = run(inputs, trace=False)
    return out
